# revision 2
# baseline (speedup 1.0000x reference)
"""Graphformer layer (full MHA) on 8 trn2 NeuronCores — v7.

Sharding: one head per core (tensor parallel over the 8 heads).

v7 restructure (vs v6):
  - sequential phases: [DMA lead + PE warmup] -> [projections, 8 chunks]
    -> [attention, 8 groups x 16 pairs].
  - ALL pairs use fp8 es + fp8 V' + DoubleRow PV (v6 had only 6/16).
  - exp alternates engines per pair: even -> ACT (Exp, fp8 out),
    odd -> DVE custom exp8 (fp8 out).
  - x chunk loads split across the two HW DGE queues (sync + scalar),
    all issued before the qk2-swap DMAs so transfers stream back-to-back.
  - PE warmup matmuls during the DMA lead-in (p-state ramp: the PE runs
    at 1.2GHz until ~3us of continuous activity).
  - V' psum->sbuf fp8 copies on the scalar engine; ot copies on scalar.
"""

from collections import deque
from contextlib import ExitStack

import numpy as np
import ml_dtypes

import concourse.bass as bass
import concourse.bacc as bacc
import concourse.mybir as mybir
from concourse.tile import TileContext, add_dep_helper

N = 4096
C = 512
D = 64
Da = D + 1
HEADS = 8
P = 128
F32 = mybir.dt.float32
BF16 = mybir.dt.bfloat16
FP8 = mybir.dt.float8e4

BF = ml_dtypes.bfloat16
F8E = ml_dtypes.float8_e4m3  # IEEE-style e4m3 (max 240) — matches TRN FP8_EXP4

F = 512  # query group width
HF = F // 2
NT = N // P  # 32 key tiles
NPAIR = NT // 2  # 16 key-tile pairs
CT = C // P  # 4 contraction tiles
G = N // F  # 8 query groups
NWARM = 7

EXP_C1 = 0.03129452
EXP_C2 = 0.00050040614
EXP_C3 = 5.012743e-06

DR_PAIRS = frozenset(range(NPAIR))  # all pairs fp8 DoubleRow
DR_IDX = {p: i for i, p in enumerate(sorted(DR_PAIRS))}
STD_TILES = sorted(mt for mt in range(NT) if (mt // 2) not in DR_PAIRS)
STD_IDX = {mt: i for i, mt in enumerate(STD_TILES)}


def _register_exp8():
    import concourse.dve_ops as dops
    from concourse.dve_ops import DveOp
    from concourse.dve_spec import Spec, Src0, C0, C1, C2, One, sq, lower
    from concourse.dve_uop import DveOpSpec

    name = "EXP8_ANT"
    for op in dops.OPS:
        if op.name == name:
            return op

    body = sq(sq(((Src0 * C2 + C1) * Src0 + C0) * Src0 + One))

    def ref(in0, in1, s0, s1, imm2):
        p = ((imm2 * in0 + s1) * in0 + s0) * in0 + 1.0
        return ((p * p) * (p * p)).astype(np.float32)

    spec = Spec(body=body, reference=ref)
    opcode = max(dops._SUB_OPCODE_FOR_NAME.values()) + 1
    assert opcode < 0x20
    dops._SUB_OPCODE_FOR_NAME[name] = opcode
    uops = lower(spec, ver="v3")
    sha = DveOpSpec(name=name, opcode=opcode, uops=uops, rd1_en=False).sha("v3")
    op = DveOp(name, spec, subdim=False, uops_sha={"v3": sha})
    dops.OPS.append(op)
    dops.CUSTOM_DVE_SPECS[name] = spec
    return op


def build_nc():
    exp8 = _register_exp8()

    nc = bacc.Bacc()
    # chunk-major layout: each chunk c is [P, CT, F] with 2KB contiguous
    # per partition in DRAM (256B lines would throttle the DMA engines)
    xTr = nc.declare_dram_parameter("xTr", [P, G, CT, F], FP8, isOutput=False)
    w1r = nc.declare_dram_parameter("w1r", [P, CT, P], BF16, isOutput=False)
    wvr = nc.declare_dram_parameter("wvr", [P, CT, D], BF16, isOutput=False)
    o = nc.declare_dram_parameter("o", [G, Da, F], F32, isOutput=True)

    with TileContext(nc) as tc, ExitStack() as ctx:
        const = ctx.enter_context(tc.tile_pool(name="const", bufs=1))
        sb = ctx.enter_context(tc.tile_pool(name="sb", bufs=1))
        esf8 = ctx.enter_context(tc.tile_pool(name="esf8", bufs=8))
        ot_pool = ctx.enter_context(tc.tile_pool(name="ot", bufs=2))
        ps_s = ctx.enter_context(tc.tile_pool(name="psS", bufs=3, space="PSUM"))
        ps_po = ctx.enter_context(tc.tile_pool(name="psPO", bufs=2, space="PSUM"))

        w1_sb = const.tile([P, CT, P], BF16, tag="w1")
        wv_sb = const.tile([P, CT, D], BF16, tag="wv")
        warm_w = const.tile([P, 80], BF16, tag="wmw")
        warm_mv = const.tile([P, F], BF16, tag="wmm")
        xt = sb.tile([P, CT, N], FP8, tag="xt")
        qk1 = sb.tile([P, N], BF16, tag="qk1")
        qk2 = sb.tile([P, N], BF16, tag="qk2")
        v8 = sb.tile([P, len(DR_PAIRS), 2, 80], FP8, tag="v8")
        nc.vector.memset(v8[:, :, :, D : D + 1], 1.0)
        nc.vector.memset(warm_w, 0.0)
        nc.vector.memset(warm_mv, 0.0)

        po = {}
        chain_prev = {"DVE": None, "ACT": None}
        pending_ot = []  # (g, po_tile) awaiting deferred evacuation

        def chain(engine, bi):
            if chain_prev[engine] is not None:
                add_dep_helper(
                    bi.ins, chain_prev[engine].ins, sync=False, reason="stream-order"
                )
            chain_prev[engine] = bi
            return bi

        def flush_ot(engine):
            """Evacuate half of a pending po -> ot; when both halves are out,
            issue the output DMA. Steady state puts both halves on ACT (DVE
            is the busier exp engine); the last group splits across engines
            and both DMA queues to shorten the kernel tail."""
            if not pending_ot:
                return
            g, pog, ot, done = pending_ot[0]
            last = g == G - 1
            half = "H0" if "H0" not in done else "H1"
            if not last:
                engine = "ACT"  # DVE is the busier exp engine in steady state
            if half == "H0":
                if engine == "ACT":
                    chain("ACT", nc.scalar.copy(out=ot[:, 0:HF], in_=pog[:, 0:HF]))
                else:
                    chain(
                        "DVE",
                        nc.vector.tensor_copy(out=ot[:, 0:HF], in_=pog[:, 0:HF]),
                    )
                if last:
                    nc.sync.dma_start(out=o[g, :, 0:HF], in_=ot[:, 0:HF])
            else:
                if engine == "ACT":
                    chain("ACT", nc.scalar.copy(out=ot[:, HF:F], in_=pog[:, HF:F]))
                else:
                    chain(
                        "DVE",
                        nc.vector.tensor_copy(out=ot[:, HF:F], in_=pog[:, HF:F]),
                    )
                if last:
                    nc.sync.dma_start(out=o[g, :, HF:F], in_=ot[:, HF:F])
            done.add(half)
            if len(done) == 2:
                if not last:
                    nc.sync.dma_start(out=o[g, :, :], in_=ot)
                pending_ot.pop(0)

        def emit_pair(g, pair, first_a_deps):
            qsl = slice(g * F, (g + 1) * F)
            mtA, mtB = 2 * pair, 2 * pair + 1
            ss = ps_s.tile([P, 2, F], F32, tag="S")
            mmA = nc.tensor.matmul(
                ss[:, 0, :],
                qk2[0:D, mtA * P : (mtA + 1) * P],
                qk1[0:D, qsl],
                start=True,
                stop=True,
            )
            for pv in first_a_deps:
                add_dep_helper(mmA.ins, pv.ins, sync=False, reason="batch-order")
            mmB = nc.tensor.matmul(
                ss[:, 1, :],
                qk1[D:P, mtB * P : (mtB + 1) * P],
                qk2[D:P, qsl],
                start=True,
                stop=True,
            )
            es = esf8.tile([P, 2, F], FP8, tag="e8")
            if g == G - 1 and pair == NPAIR - 1:
                # final pair: split the exp across both engines to shorten
                # the kernel tail (DVE emitted first)
                chain(
                    "DVE",
                    nc.vector._custom_dve(
                        exp8,
                        out=es[:, 1, :],
                        in0=ss[:, 1, :],
                        s0=EXP_C1,
                        s1=EXP_C2,
                        imm2=EXP_C3,
                    ),
                )
                chain(
                    "ACT",
                    nc.scalar.activation(
                        out=es[:, 0, :],
                        in_=ss[:, 0, :],
                        func=mybir.ActivationFunctionType.Exp,
                        scale=0.125,
                    ),
                )
            elif pair % 2 == 1:
                chain(
                    "DVE",
                    nc.vector._custom_dve(
                        exp8, out=es, in0=ss, s0=EXP_C1, s1=EXP_C2, imm2=EXP_C3
                    ),
                )
            else:
                chain(
                    "ACT",
                    nc.scalar.activation(
                        out=es,
                        in_=ss,
                        func=mybir.ActivationFunctionType.Exp,
                        scale=0.125,
                    ),
                )
                flush_ot("ACT")
            return mmB, es

        def emit_pv(g, pair, es, last_mmB):
            if pair == 0:
                po[g] = ps_po.tile([Da, F], F32, tag="po", name=f"po{g}")
            mm = nc.tensor.matmul(
                po[g],
                v8[:, DR_IDX[pair], :, 0:Da],
                es,
                start=(pair == 0),
                stop=(pair == NPAIR - 1),
                perf_mode=mybir.MatmulPerfMode.DoubleRow,
            )
            if last_mmB is not None:
                add_dep_helper(mm.ins, last_mmB.ins, sync=False, reason="batch-order")
            if pair == NPAIR - 1:
                ot = ot_pool.tile([Da, F], F32, tag="ot", name=f"ot{g}")
                pending_ot.append((g, po.pop(g), ot, set()))
                if g == G - 1:
                    # last group: evacuate immediately on both engines
                    flush_ot("ACT")
                    flush_ot("DVE")
            return mm

        # ---- DMA issue: weights + all x chunks, alternating between the
        # two HW DGE queues (sync / scalar) so transfers stream
        # back-to-back on both. Scalar-queue issues burn ~700ns of ACT
        # sequencer time each, so scalar only carries w1 + 4 x chunks;
        # everything else (incl. all swaps and outputs) issues from Sync.
        nc.scalar.dma_start(out=w1_sb, in_=w1r[:, :, :])
        nc.sync.dma_start(out=xt[:, :, 0:HF], in_=xTr[:, 0, :, 0:HF])
        nc.scalar.dma_start(out=xt[:, :, HF:F], in_=xTr[:, 0, :, HF:F])
        nc.sync.dma_start(out=wv_sb, in_=wvr[:, :, :])
        for c in range(1, G):
            sl = slice(c * F, (c + 1) * F)
            eng = nc.scalar if c % 2 == 1 else nc.sync
            eng.dma_start(out=xt[:, :, sl], in_=xTr[:, c, :, :])

        # ---- PE warmup: garbage matmuls (zero weights) to start the
        # p-state ramp while the first x chunk is still in flight.
        wm_po = ps_po.tile([Da, F], F32, tag="po", name="warm")
        for i in range(NWARM):
            nc.tensor.matmul(
                wm_po, warm_w[:, 0:Da], warm_mv, start=True, stop=True
            )

        # ---- phase P: projections (all 8 chunks; no attention yet)
        q_swaps = []
        for c in range(G):
            sl = slice(c * F, (c + 1) * F)
            pp = ps_s.tile([P, F], F32, tag="S", name=f"pp{c}")
            for ci in range(CT):
                nc.tensor.matmul(
                    pp,
                    w1_sb[:, ci, :],
                    xt[:, ci, sl],
                    start=(ci == 0),
                    stop=(ci == CT - 1),
                )
            chain("DVE", nc.vector.tensor_copy(out=qk1[:, sl], in_=pp))
            # K-half swap: feeds every group's QK lhsT, needed from
            # attention start; Q-half swap of chunk c is only read once
            # group c begins (attn_start + 10.4us*c), so defer c>=1 to
            # after the x stream. All on the Sync queue (its sequencer
            # is otherwise idle).
            nc.sync.dma_start(out=qk2[0:D, sl], in_=qk1[D:P, sl])
            if c == 0:
                nc.sync.dma_start(out=qk2[D:P, sl], in_=qk1[0:D, sl])
            else:
                q_swaps.append((c, sl))
            # all 4 V-proj tiles of the chunk in ONE psum allocation, so the
            # shared S-ring recycles once per chunk instead of 5x
            pv4 = ps_s.tile([P, 4, D], F32, tag="S", name=f"pv4_{c}")
            for j in range(4):
                mt = c * (F // P) + j
                for ci in range(CT):
                    nc.tensor.matmul(
                        pv4[:, j, :],
                        xt[:, ci, mt * P : (mt + 1) * P],
                        wv_sb[:, ci, :],
                        start=(ci == 0),
                        stop=(ci == CT - 1),
                    )
            for k in range(2):
                pair = 2 * c + k
                chain(
                    "ACT",
                    nc.scalar.copy(
                        out=v8[:, DR_IDX[pair], :, 0:D],
                        in_=pv4[:, 2 * k : 2 * k + 2, :],
                    ),
                )

        # deferred Q-half swaps (transfer during early attention)
        for c, sl in q_swaps:
            nc.sync.dma_start(out=qk2[D:P, sl], in_=qk1[0:D, sl])

        # ---- phase ATT: groups 0-7, 16 pairs each, emitted in batches so
        # the PE stays in one tiling/perf mode for several matmuls at a
        # time (each 64x128-row-tile <-> DoubleRow mode switch drains the
        # array, ~100ns). PV matmuls for batch k are emitted after batch
        # k+1's QK matmuls (deferred one batch) so their es is ready.
        BATCH = [3, 3, 3, 3, 2, 2]
        assert sum(BATCH) == NPAIR
        deferred = []  # (g, pair, es) whose PV is not yet emitted
        prev_pvs = []  # PV matmuls of the last flushed batch
        for g in range(G):
            p0 = 0
            for bsz in BATCH:
                cur = []
                last_mmB = None
                for pr in range(p0, p0 + bsz):
                    mmB, es = emit_pair(g, pr, prev_pvs if pr == p0 else [])
                    last_mmB = mmB
                    cur.append((g, pr, es))
                prev_pvs = [
                    emit_pv(gg, pp, ee, last_mmB) for (gg, pp, ee) in deferred
                ]
                deferred = cur
                p0 += bsz
        # final batch's PVs have no following QK batch
        for gg, pp, ee in deferred:
            emit_pv(gg, pp, ee, None)
    nc.compile()
    return nc


def make_in_maps(x, Wq, Wk, Wv, Wo):
    x = np.asarray(x, dtype=np.float32)
    Wq = np.asarray(Wq, dtype=np.float32)
    Wk = np.asarray(Wk, dtype=np.float32)
    Wv = np.asarray(Wv, dtype=np.float32)
    Wo = np.asarray(Wo, dtype=np.float32)
    xT = np.ascontiguousarray(x.T)  # [C, N]
    # [P, G, CT, F]: chunk-major so each chunk's DMA reads 2KB contiguous
    # per partition from DRAM
    xTr = np.ascontiguousarray(
        xT.reshape(CT, P, G, F).transpose(1, 2, 0, 3)
    ).astype(F8E)
    in_maps = []
    for h in range(HEADS):
        sl = slice(h * D, (h + 1) * D)
        wqk = np.concatenate([Wq[sl].T, Wk[sl].T], axis=1)  # [C, 128]
        w1r = np.ascontiguousarray(
            wqk.reshape(CT, P, P).transpose(1, 0, 2)
        ).astype(BF)
        wprime = (Wo[:, sl] @ Wv[sl]).T  # [C, D]
        wvr = np.ascontiguousarray(
            wprime.reshape(CT, P, D).transpose(1, 0, 2)
        ).astype(BF)
        in_maps.append({"xTr": xTr, "w1r": w1r, "wvr": wvr})
    return in_maps


_CACHE = {}


def run_on_hw(x, Wq, Wk, Wv, Wo, bo, trace=False):
    from concourse.bass_utils import run_bass_kernel_spmd

    if "nc" not in _CACHE:
        _CACHE["nc"] = build_nc()
    nc = _CACHE["nc"]
    in_maps = make_in_maps(x, Wq, Wk, Wv, Wo)
    res = run_bass_kernel_spmd(nc, in_maps, list(range(HEADS)), trace=trace)
    out = np.zeros((N, D), np.float32)
    for r in res.results:
        og = r["o"]  # [G, Da, F]
        num = og[:, 0:D, :]
        den = og[:, D, :]
        yc = (num / den[:, None, :]).transpose(0, 2, 1).reshape(N, D)
        out += yc
    out += np.asarray(bo, dtype=np.float32)[None, :]
    return out, res


def kernel(x, Wq, Wk, Wv, Wo, bo):
    out, _ = run_on_hw(x, Wq, Wk, Wv, Wo, bo)
    return out


# revision 3
# speedup vs baseline: 1.0262x; 1.0262x over previous
"""Graphformer layer (full MHA) on 8 trn2 NeuronCores — v8.

Sharding: one head per core (tensor parallel over the 8 heads).
Factorization: [Wq;Wk] fused projection; Wo folded into V (V' = x @ (Wo_h Wv_h)^T);
softmax denominator via a ones-column in V' (Da = 65); host divides + sums heads.

Structure (vs the v6 baseline):
  - sequential phases: [DMA lead + PE warmup] -> [projections, 8 chunks]
    -> [attention, 8 groups x 16 key-tile pairs].
  - x is loaded as fp8e4m3 (2MB instead of 4MB; DMA-bound phase), laid out
    chunk-major in DRAM so every chunk reads 2KB contiguous per partition.
  - ALL 16 pairs use fp8 es + fp8 V' + DoubleRow PV (v6 had 6/16).
  - exp alternates engines per pair: even -> ACT (Exp, fp8 out directly),
    odd -> DVE custom exp8 (fp8 out). Both engines run ~90% busy.
  - attention emitted in batches of 3 pairs (QKx3 then PVx3, PVs deferred
    one batch): the PE drains on every 64x128-row-tile <-> DoubleRow mode
    switch, so batching saves ~100ns per avoided switch.
  - po evacuation (ot) deferred into the next group's first ACT slots;
    last group splits it across both engines + both DMA queues.
  - qk2 K-half swaps early; Q-half swaps deferred (group g only reads its
    own Q-swap ~10.4us*g into attention).
  - PE warmup matmuls during the DMA lead-in (p-state: the PE runs at
    1.2GHz until ~3us of continuous activity).
"""

from contextlib import ExitStack

import numpy as np
import ml_dtypes

import concourse.bacc as bacc
import concourse.mybir as mybir
from concourse.tile import TileContext, add_dep_helper

N = 4096
C = 512
D = 64
Da = D + 1
HEADS = 8
P = 128
F32 = mybir.dt.float32
BF16 = mybir.dt.bfloat16
FP8 = mybir.dt.float8e4

BF = ml_dtypes.bfloat16
F8E = ml_dtypes.float8_e4m3  # IEEE-style e4m3 (max 240) — matches TRN FP8_EXP4

F = 512  # query group width
HF = F // 2
NT = N // P  # 32 key tiles
NPAIR = NT // 2  # 16 key-tile pairs
CT = C // P  # 4 contraction tiles
G = N // F  # 8 query groups
NWARM = 7

EXP_C1 = 0.03129452
EXP_C2 = 0.00050040614
EXP_C3 = 5.012743e-06

DR_PAIRS = frozenset(range(NPAIR))  # all pairs fp8 DoubleRow
DR_IDX = {p: i for i, p in enumerate(sorted(DR_PAIRS))}


def _register_exp8():
    import concourse.dve_ops as dops
    from concourse.dve_ops import DveOp
    from concourse.dve_spec import Spec, Src0, C0, C1, C2, One, sq, lower
    from concourse.dve_uop import DveOpSpec

    name = "EXP8_ANT"
    for op in dops.OPS:
        if op.name == name:
            return op

    body = sq(sq(((Src0 * C2 + C1) * Src0 + C0) * Src0 + One))

    def ref(in0, in1, s0, s1, imm2):
        p = ((imm2 * in0 + s1) * in0 + s0) * in0 + 1.0
        return ((p * p) * (p * p)).astype(np.float32)

    spec = Spec(body=body, reference=ref)
    opcode = max(dops._SUB_OPCODE_FOR_NAME.values()) + 1
    assert opcode < 0x20
    dops._SUB_OPCODE_FOR_NAME[name] = opcode
    uops = lower(spec, ver="v3")
    sha = DveOpSpec(name=name, opcode=opcode, uops=uops, rd1_en=False).sha("v3")
    op = DveOp(name, spec, subdim=False, uops_sha={"v3": sha})
    dops.OPS.append(op)
    dops.CUSTOM_DVE_SPECS[name] = spec
    return op


def build_nc():
    exp8 = _register_exp8()

    nc = bacc.Bacc()
    # chunk-major layout: each chunk c is [P, CT, F] with 2KB contiguous
    # per partition in DRAM (256B lines would throttle the DMA engines)
    xTr = nc.declare_dram_parameter("xTr", [P, G, CT, F], FP8, isOutput=False)
    w1r = nc.declare_dram_parameter("w1r", [P, CT, P], BF16, isOutput=False)
    wvr = nc.declare_dram_parameter("wvr", [P, CT, D], BF16, isOutput=False)
    o = nc.declare_dram_parameter("o", [G, Da, F], F32, isOutput=True)

    with TileContext(nc) as tc, ExitStack() as ctx:
        const = ctx.enter_context(tc.tile_pool(name="const", bufs=1))
        sb = ctx.enter_context(tc.tile_pool(name="sb", bufs=1))
        esf8 = ctx.enter_context(tc.tile_pool(name="esf8", bufs=8))
        ot_pool = ctx.enter_context(tc.tile_pool(name="ot", bufs=2))
        ps_s = ctx.enter_context(tc.tile_pool(name="psS", bufs=3, space="PSUM"))
        ps_po = ctx.enter_context(tc.tile_pool(name="psPO", bufs=2, space="PSUM"))

        w1_sb = const.tile([P, CT, P], BF16, tag="w1")
        wv_sb = const.tile([P, CT, D], BF16, tag="wv")
        warm_w = const.tile([P, 80], BF16, tag="wmw")
        warm_mv = const.tile([P, F], BF16, tag="wmm")
        xt = sb.tile([P, CT, N], FP8, tag="xt")
        qk1 = sb.tile([P, N], BF16, tag="qk1")
        qk2 = sb.tile([P, N], BF16, tag="qk2")
        v8 = sb.tile([P, len(DR_PAIRS), 2, 80], FP8, tag="v8")
        nc.vector.memset(v8[:, :, :, D : D + 1], 1.0)
        nc.vector.memset(warm_w, 0.0)
        nc.vector.memset(warm_mv, 0.0)

        po = {}
        chain_prev = {"DVE": None, "ACT": None}
        pending_ot = []  # (g, po_tile) awaiting deferred evacuation

        def chain(engine, bi):
            if chain_prev[engine] is not None:
                add_dep_helper(
                    bi.ins, chain_prev[engine].ins, sync=False, reason="stream-order"
                )
            chain_prev[engine] = bi
            return bi

        def flush_ot(engine):
            """Evacuate half of a pending po -> ot; when both halves are out,
            issue the output DMA. Steady state puts both halves on ACT (DVE
            is the busier exp engine); the last group splits across engines
            and both DMA queues to shorten the kernel tail."""
            if not pending_ot:
                return
            g, pog, ot, done = pending_ot[0]
            last = g == G - 1
            half = "H0" if "H0" not in done else "H1"
            if not last:
                engine = "ACT"  # DVE is the busier exp engine in steady state
            if half == "H0":
                if engine == "ACT":
                    chain("ACT", nc.scalar.copy(out=ot[:, 0:HF], in_=pog[:, 0:HF]))
                else:
                    chain(
                        "DVE",
                        nc.vector.tensor_copy(out=ot[:, 0:HF], in_=pog[:, 0:HF]),
                    )
                if last:
                    nc.sync.dma_start(out=o[g, :, 0:HF], in_=ot[:, 0:HF])
            else:
                if engine == "ACT":
                    chain("ACT", nc.scalar.copy(out=ot[:, HF:F], in_=pog[:, HF:F]))
                else:
                    chain(
                        "DVE",
                        nc.vector.tensor_copy(out=ot[:, HF:F], in_=pog[:, HF:F]),
                    )
                if last:
                    nc.sync.dma_start(out=o[g, :, HF:F], in_=ot[:, HF:F])
            done.add(half)
            if len(done) == 2:
                if not last:
                    nc.sync.dma_start(out=o[g, :, :], in_=ot)
                pending_ot.pop(0)

        def emit_pair(g, pair, first_a_deps):
            qsl = slice(g * F, (g + 1) * F)
            mtA, mtB = 2 * pair, 2 * pair + 1
            ss = ps_s.tile([P, 2, F], F32, tag="S")
            mmA = nc.tensor.matmul(
                ss[:, 0, :],
                qk2[0:D, mtA * P : (mtA + 1) * P],
                qk1[0:D, qsl],
                start=True,
                stop=True,
            )
            for pv in first_a_deps:
                add_dep_helper(mmA.ins, pv.ins, sync=False, reason="batch-order")
            mmB = nc.tensor.matmul(
                ss[:, 1, :],
                qk1[D:P, mtB * P : (mtB + 1) * P],
                qk2[D:P, qsl],
                start=True,
                stop=True,
            )
            es = esf8.tile([P, 2, F], FP8, tag="e8")
            if g == G - 1 and pair == NPAIR - 1:
                # final pair: split the exp across both engines to shorten
                # the kernel tail (DVE emitted first)
                chain(
                    "DVE",
                    nc.vector._custom_dve(
                        exp8,
                        out=es[:, 1, :],
                        in0=ss[:, 1, :],
                        s0=EXP_C1,
                        s1=EXP_C2,
                        imm2=EXP_C3,
                    ),
                )
                chain(
                    "ACT",
                    nc.scalar.activation(
                        out=es[:, 0, :],
                        in_=ss[:, 0, :],
                        func=mybir.ActivationFunctionType.Exp,
                        scale=0.125,
                    ),
                )
            elif pair % 2 == 1:
                chain(
                    "DVE",
                    nc.vector._custom_dve(
                        exp8, out=es, in0=ss, s0=EXP_C1, s1=EXP_C2, imm2=EXP_C3
                    ),
                )
            else:
                chain(
                    "ACT",
                    nc.scalar.activation(
                        out=es,
                        in_=ss,
                        func=mybir.ActivationFunctionType.Exp,
                        scale=0.125,
                    ),
                )
                flush_ot("ACT")
            return mmB, es

        def emit_pv(g, pair, es, last_mmB):
            if pair == 0:
                po[g] = ps_po.tile([Da, F], F32, tag="po", name=f"po{g}")
            mm = nc.tensor.matmul(
                po[g],
                v8[:, DR_IDX[pair], :, 0:Da],
                es,
                start=(pair == 0),
                stop=(pair == NPAIR - 1),
                perf_mode=mybir.MatmulPerfMode.DoubleRow,
            )
            if last_mmB is not None:
                add_dep_helper(mm.ins, last_mmB.ins, sync=False, reason="batch-order")
            if pair == NPAIR - 1:
                ot = ot_pool.tile([Da, F], F32, tag="ot", name=f"ot{g}")
                pending_ot.append((g, po.pop(g), ot, set()))
                if g == G - 1:
                    # last group: evacuate immediately on both engines
                    flush_ot("ACT")
                    flush_ot("DVE")
            return mm

        # ---- DMA issue: weights + all x chunks, alternating between the
        # two HW DGE queues (sync / scalar) so transfers stream
        # back-to-back on both. Scalar-queue issues burn ~700ns of ACT
        # sequencer time each, so scalar only carries w1 + 4 x chunks;
        # everything else (incl. all swaps and outputs) issues from Sync.
        nc.scalar.dma_start(out=w1_sb, in_=w1r[:, :, :])
        nc.sync.dma_start(out=xt[:, :, 0:HF], in_=xTr[:, 0, :, 0:HF])
        nc.scalar.dma_start(out=xt[:, :, HF:F], in_=xTr[:, 0, :, HF:F])
        nc.sync.dma_start(out=wv_sb, in_=wvr[:, :, :])
        for c in range(1, G):
            sl = slice(c * F, (c + 1) * F)
            eng = nc.scalar if c % 2 == 1 else nc.sync
            eng.dma_start(out=xt[:, :, sl], in_=xTr[:, c, :, :])

        # ---- PE warmup: garbage matmuls (zero weights) to start the
        # p-state ramp while the first x chunk is still in flight.
        wm_po = ps_po.tile([Da, F], F32, tag="po", name="warm")
        for i in range(NWARM):
            nc.tensor.matmul(
                wm_po, warm_w[:, 0:Da], warm_mv, start=True, stop=True
            )

        # ---- phase P: projections (all 8 chunks; no attention yet)
        q_swaps = []
        for c in range(G):
            sl = slice(c * F, (c + 1) * F)
            pp = ps_s.tile([P, F], F32, tag="S", name=f"pp{c}")
            for ci in range(CT):
                nc.tensor.matmul(
                    pp,
                    w1_sb[:, ci, :],
                    xt[:, ci, sl],
                    start=(ci == 0),
                    stop=(ci == CT - 1),
                )
            chain("DVE", nc.vector.tensor_copy(out=qk1[:, sl], in_=pp))
            # K-half swap: feeds every group's QK lhsT, needed from
            # attention start; Q-half swap of chunk c is only read once
            # group c begins (attn_start + 10.4us*c), so defer c>=1 to
            # after the x stream. All on the Sync queue (its sequencer
            # is otherwise idle).
            nc.sync.dma_start(out=qk2[0:D, sl], in_=qk1[D:P, sl])
            if c == 0:
                nc.sync.dma_start(out=qk2[D:P, sl], in_=qk1[0:D, sl])
            else:
                q_swaps.append((c, sl))
            # all 4 V-proj tiles of the chunk in ONE psum allocation, so the
            # shared S-ring recycles once per chunk instead of 5x
            pv4 = ps_s.tile([P, 4, D], F32, tag="S", name=f"pv4_{c}")
            for j in range(4):
                mt = c * (F // P) + j
                for ci in range(CT):
                    nc.tensor.matmul(
                        pv4[:, j, :],
                        xt[:, ci, mt * P : (mt + 1) * P],
                        wv_sb[:, ci, :],
                        start=(ci == 0),
                        stop=(ci == CT - 1),
                    )
            for k in range(2):
                pair = 2 * c + k
                chain(
                    "ACT",
                    nc.scalar.copy(
                        out=v8[:, DR_IDX[pair], :, 0:D],
                        in_=pv4[:, 2 * k : 2 * k + 2, :],
                    ),
                )

        # deferred Q-half swaps (transfer during early attention)
        for c, sl in q_swaps:
            nc.sync.dma_start(out=qk2[D:P, sl], in_=qk1[0:D, sl])

        # ---- phase ATT: groups 0-7, 16 pairs each, emitted in batches so
        # the PE stays in one tiling/perf mode for several matmuls at a
        # time (each 64x128-row-tile <-> DoubleRow mode switch drains the
        # array, ~100ns). PV matmuls for batch k are emitted after batch
        # k+1's QK matmuls (deferred one batch) so their es is ready.
        BATCH = [3, 3, 3, 3, 2, 2]
        assert sum(BATCH) == NPAIR
        deferred = []  # (g, pair, es) whose PV is not yet emitted
        prev_pvs = []  # PV matmuls of the last flushed batch
        for g in range(G):
            p0 = 0
            for bsz in BATCH:
                cur = []
                last_mmB = None
                for pr in range(p0, p0 + bsz):
                    mmB, es = emit_pair(g, pr, prev_pvs if pr == p0 else [])
                    last_mmB = mmB
                    cur.append((g, pr, es))
                prev_pvs = [
                    emit_pv(gg, pp, ee, last_mmB) for (gg, pp, ee) in deferred
                ]
                deferred = cur
                p0 += bsz
        # final batch's PVs have no following QK batch
        for gg, pp, ee in deferred:
            emit_pv(gg, pp, ee, None)
    nc.compile()
    return nc


def make_in_maps(x, Wq, Wk, Wv, Wo):
    x = np.asarray(x, dtype=np.float32)
    Wq = np.asarray(Wq, dtype=np.float32)
    Wk = np.asarray(Wk, dtype=np.float32)
    Wv = np.asarray(Wv, dtype=np.float32)
    Wo = np.asarray(Wo, dtype=np.float32)
    xT = np.ascontiguousarray(x.T)  # [C, N]
    # [P, G, CT, F]: chunk-major so each chunk's DMA reads 2KB contiguous
    # per partition from DRAM
    xTr = np.ascontiguousarray(
        xT.reshape(CT, P, G, F).transpose(1, 2, 0, 3)
    ).astype(F8E)
    in_maps = []
    for h in range(HEADS):
        sl = slice(h * D, (h + 1) * D)
        wqk = np.concatenate([Wq[sl].T, Wk[sl].T], axis=1)  # [C, 128]
        w1r = np.ascontiguousarray(
            wqk.reshape(CT, P, P).transpose(1, 0, 2)
        ).astype(BF)
        wprime = (Wo[:, sl] @ Wv[sl]).T  # [C, D]
        wvr = np.ascontiguousarray(
            wprime.reshape(CT, P, D).transpose(1, 0, 2)
        ).astype(BF)
        in_maps.append({"xTr": xTr, "w1r": w1r, "wvr": wvr})
    return in_maps


_CACHE = {}


def run_on_hw(x, Wq, Wk, Wv, Wo, bo, trace=False):
    from concourse.bass_utils import run_bass_kernel_spmd

    if "nc" not in _CACHE:
        _CACHE["nc"] = build_nc()
    nc = _CACHE["nc"]
    in_maps = make_in_maps(x, Wq, Wk, Wv, Wo)
    res = run_bass_kernel_spmd(nc, in_maps, list(range(HEADS)), trace=trace)
    out = np.zeros((N, D), np.float32)
    for r in res.results:
        og = r["o"]  # [G, Da, F]
        num = og[:, 0:D, :]
        den = og[:, D, :]
        yc = (num / den[:, None, :]).transpose(0, 2, 1).reshape(N, D)
        out += yc
    out += np.asarray(bo, dtype=np.float32)[None, :]
    return out, res


def kernel(x, Wq, Wk, Wv, Wo, bo):
    out, _ = run_on_hw(x, Wq, Wk, Wv, Wo, bo)
    return out
